# revision 55
# baseline (speedup 1.0000x reference)
"""GAT/GCN message-passing layer on 8 Trainium2 NeuronCores.

Math: the reference computes, per query node i,
    e[i,j]   = f_src[i] + f_dst[j]           (GAT additive attention, masked by Ahat>0)
    attn     = softmax_j(e masked)
    out      = relu(attn @ h_prime)
The f_src[i] term is constant along the softmax axis, so it cancels:
    attn[i,j] = Ahat[i,j]*exp(f_dst[j]) / sum_k Ahat[i,k]*exp(f_dst[k])
With g = exp(f_dst) the whole layer is one GEMM over the adjacency:
    out = relu( (Ahat @ [g*h_prime | g])[:, :256] / (Ahat @ [g*h_prime | g])[:, 256:] )
where h_prime = node_feats @ w and f_dst = node_feats @ (w @ w_a @ a[2:4]).

Sharding: 1D row partition of query nodes.  Each of the 8 cores owns 1024 rows
of Ahat (fed pre-transposed, [8192, 1024], so the contraction axis j lands on
SBUF partitions) and computes its 1024x256 slice of the output.  The small
tensors (node_feats^T, [w | u]) are replicated; each core recomputes the
B = [g*h_prime | g] panel locally, which is cheaper than a collective.

The adjacency is binary so bf16 is lossless for it; everything else is bf16 in
/ fp32-PSUM accumulate (measured end-to-end rel-err ~2e-3 vs fp32 reference).

A quirk this kernel works around everywhere: this walrus accepts only ONE sync
wait per instruction, so the dataflow is arranged so every instruction has at
most one cross-engine dependency (single-engine op chains, ACT-issued DMAs
that inherit the scalar sequencer's observed clock, and read/write "absorber"
ops ahead of DMAs that would otherwise need two waits).
"""

import os
import sys

import numpy as np

sys.path.insert(0, "/opt/trn_rl_repo")

import concourse.bass as bass  # noqa: E402
import concourse.tile as tile  # noqa: E402
from concourse import mybir  # noqa: E402
from concourse.bass_utils import run_bass_kernel_spmd  # noqa: E402
from concourse.tile import add_dep_helper  # noqa: E402

N = 8192
F = 256  # in_features == out_features
FE = F + 1  # h_prime columns + the g column
NCORES = 8
ROWS = N // NCORES  # 1024 output rows per core
P = 128
NJ = N // P  # 64 contraction blocks
NI = ROWS // P  # 8 output-row blocks per core

BF = mybir.dt.bfloat16

# j-block layout of the adjacency stream:
#   pinned tiles: j-blocks 0..JSTART-1 stay resident (PJ j-blocks per tile)
#   rotating ring: j-blocks JSTART..63 through NBUF slots (CJ j-blocks per DMA)
PJ = 8
NPIN = 4
JSTART = NPIN * PJ  # 32
CJ = 2
NROT = (NJ - JSTART) // CJ  # 16
NBUF = 8

_CACHE = {}


def _build():
    nc = bass.Bass(
        "TRN2",
        target_bir_lowering=False,
        debug=False,
        enable_asserts=True,
        num_devices=NCORES,
    )
    aT = nc.dram_tensor("aT", [N, ROWS], BF, kind="ExternalInput").ap()
    # params = [w | u | node_feats^T], all bf16, loaded in column chunks
    params = nc.dram_tensor("params", [F, FE + N], BF, kind="ExternalInput").ap()
    out = nc.dram_tensor("out", [ROWS, F], mybir.dt.float32, kind="ExternalOutput").ap()

    with tile.TileContext(nc) as tc:
        _body(tc, aT, params, out)
    return nc


def _body(tc, aT, params, out):
    nc = tc.nc
    f32 = mybir.dt.float32
    Exp = mybir.ActivationFunctionType.Exp

    with (
        tc.tile_pool(name="consts", bufs=1) as consts,
        tc.tile_pool(name="apool", bufs=1) as apool,
        tc.tile_pool(name="gpool", bufs=4) as gpool,
        tc.tile_pool(name="opool", bufs=1) as opool,
        tc.tile_pool(name="rpool", bufs=8) as rpool,
        tc.tile_pool(name="psum", bufs=1, space="PSUM") as psum,
    ):
        # ---- loads ---------------------------------------------------------
        # params in NCHP column chunks, ACT-issued so the prefix matmuls'
        # dependencies (chunk DMA + PSUM-bank WAR + B reads) all collapse onto
        # the single Activation semaphore.
        params_sb = consts.tile([P, 2, FE + N], BF, tag="params")
        # 3 chunks: each ACT DMA issue costs ~1.9us of scalar-sequencer time
        # (256 descriptors), so few chunks -- but a small first chunk (wext +
        # 8 j-blocks) lets the prefix matmuls start at ~4us.
        NCHP = 3
        bounds = [0, FE + 512, FE + 4096, FE + N]
        pchunks = []
        for c in range(NCHP):
            lo, hi = bounds[c], bounds[c + 1]
            pchunks.append(
                nc.scalar.dma_start(
                    params_sb[:, :, lo:hi],
                    params[:, lo:hi].rearrange("(o p) f -> p o f", p=P),
                )
            )
        wext_sb = params_sb[:, :, 0:FE]
        nfT_sb = [params_sb[:, kb, FE : FE + N] for kb in range(2)]

        # Adjacency: 4 pinned 4MB loads (j-blocks 0..31) + rotating ring.
        # The adjacency loads (20MB) would otherwise starve the params
        # chunks on the shared SDMA fabric and delay the first prefix matmul
        # to ~16us; gate them behind the params data (not needed until ~35us).
        pinned = []
        for t in range(NPIN):
            pt = consts.tile([P, PJ * ROWS], BF, tag=f"aTp{t}", name=f"aTp{t}")
            pinned.append(pt)
            pdma = nc.sync.dma_start(
                pt.rearrange("p (o f) -> p o f", f=ROWS),
                aT[t * PJ * P : (t + 1) * PJ * P, :].rearrange(
                    "(o p) f -> p o f", p=P
                ),
            )
            add_dep_helper(pdma.ins, pchunks[0 if t == 0 else 2].ins, reason="params first")
        rot = []
        for rc in range(NBUF):
            rt = apool.tile([P, CJ * ROWS], BF, tag=f"aTr{rc % NBUF}", name=f"aTr{rc}")
            rot.append(rt)
            rdma = nc.sync.dma_start(
                rt.rearrange("p (o f) -> p o f", f=ROWS),
                aT[(JSTART + rc * CJ) * P : (JSTART + (rc + 1) * CJ) * P, :].rearrange(
                    "(o p) f -> p o f", p=P
                ),
            )
            add_dep_helper(rdma.ins, pchunks[2].ins, reason="params first")

        def a_block(j):
            """SBUF [128, 128] lhsT view of adjacency j-block, i-block i."""
            if j < JSTART:
                t = pinned[j // PJ]
                o = j % PJ
            else:
                t = rot[(j - JSTART) // CJ]
                o = (j - JSTART) % CJ
            return t, o

        # ---- PSUM accumulators --------------------------------------------
        acc = [
            psum.tile([P, FE], f32, tag=f"acc{i}", name=f"acc{i}") for i in range(NI)
        ]

        # ---- prefix: B[j] = [g*h_prime | g], all 64 j-blocks ---------------
        # h' matmuls borrow PSUM banks 0/1; those banks' accumulation groups
        # start at j=JSTART in the main stream (their earlier j-terms are
        # backfilled at the end from the pinned tiles) so the bank WAR
        # against the last B-prep doesn't stall the in-order PE queue.
        B_all = consts.tile([P, NJ * FE], BF, tag="B")
        btile = [B_all[:, j * FE : (j + 1) * FE] for j in range(NJ)]
        G = consts.tile([P, NJ], f32, tag="G")
        prev_act = None
        for j in range(NJ):
            hp = acc[j % 2]
            for kb in range(2):
                nc.tensor.matmul(
                    hp[:],
                    lhsT=nfT_sb[kb][:, j * P : (j + 1) * P],
                    rhs=wext_sb[:, kb, :],
                    start=(kb == 0),
                    stop=(kb == 1),
                )
            b = btile[j]
            gj = G[:, j : j + 1]
            ex = nc.scalar.activation(gj, hp[:, F : F + 1], Exp)
            if prev_act is not None:
                # keep B-prep in emission order on ACT; a scheduler shuffle
                # makes some of these ops pick up a second (same-sem) wait
                add_dep_helper(ex.ins, prev_act.ins, sync=False, reason="act order")
            prev_act = nc.scalar.mul(b[:, 0:F], hp[:, 0:F], gj)
            if j % 8 == 7:
                # one strided cast-copy drops this 8-group's g column into B
                c0 = j - 7
                prev_act = nc.scalar.copy(
                    B_all[:, c0 * FE + F : (j + 1) * FE : FE], G[:, c0 : j + 1]
                )

        # ---- main stream ---------------------------------------------------
        last_mm = None
        refills = []
        scr_last = None
        for j in range(NJ):
            t, o = a_block(j)
            ilist = range(NI) if j >= JSTART else range(2, NI)
            for i in ilist:
                first = j == 0 or (j == JSTART and i < 2)
                last_mm = nc.tensor.matmul(
                    acc[i][:],
                    lhsT=t[:, o * ROWS + i * P : o * ROWS + (i + 1) * P],
                    rhs=btile[j][:],
                    start=first,
                    stop=(j == NJ - 1 and i >= 2),
                )
            # ring refill bookkeeping (rotating region only)
            if j >= JSTART and (j - JSTART) % CJ == CJ - 1:
                rc = (j - JSTART) // CJ
                nxt = rc + NBUF
                if nxt < NROT:
                    at = rot[rc]
                    # single-wait absorbers: ACT read (old DMA dep) + ACT
                    # zero-write (PE readers dep); ACT-issued refill then
                    # needs only one wait.
                    head = at[:, 0:1]
                    tail = at[:, CJ * ROWS - 1 : CJ * ROWS]
                    scr = gpool.tile([P, 1], BF, tag="scr")
                    scr_last = scr
                    rd = nc.scalar.copy(scr[:], head)
                    wz = nc.scalar.mul(tail, tail, 0.0)
                    add_dep_helper(wz.ins, rd.ins, sync=False, reason="rd<wz")
                    at_new = apool.tile(
                        [P, CJ * ROWS], BF, tag=f"aTr{nxt % NBUF}", name=f"aTr{nxt}"
                    )
                    rot.append(at_new)
                    refill = nc.scalar.dma_start(
                        at_new.rearrange("p (o f) -> p o f", f=ROWS),
                        aT[
                            (JSTART + nxt * CJ) * P : (JSTART + (nxt + 1) * CJ) * P, :
                        ].rearrange("(o p) f -> p o f", p=P),
                    )
                    add_dep_helper(refill.ins, wz.ins, sync=False, reason="rf<wz")
                    refills.append(refill)

        # backfill: banks 0/1 take their j<JSTART terms from the pinned tiles
        for j in range(JSTART):
            t, o = a_block(j)
            for i in range(2):
                last_mm = nc.tensor.matmul(
                    acc[i][:],
                    lhsT=t[:, o * ROWS + i * P : o * ROWS + (i + 1) * P],
                    rhs=btile[j][:],
                    start=False,
                    stop=(j == JSTART - 1),
                )

        # ---- epilogue: out[i] = relu(acc[i][:, :F] / acc[i][:, F]) ---------
        # split into banks 2..7 (stop at main j=63, drain while the backfill
        # matmuls still run) and banks 0/1 (stop at backfill end).  ACT copies
        # the denominators into SBUF (one PE wait each, avoids DVE-reads-PSUM
        # bank deps), DVE does reciprocal / scale / relu, SWDGE stores.
        otile = opool.tile([P, NI * F], f32, tag="o")
        stores = []
        denom_last = None
        last_dve = None

        def epi(banks, tag):
            nonlocal denom_last, last_dve
            nb = len(banks)
            denom = rpool.tile([P, nb], f32, tag=f"denom{tag}", name=f"denom{tag}")
            for k, i in enumerate(banks):
                dc = nc.scalar.copy(denom[:, k : k + 1], acc[i][:, F : F + 1])
                if denom_last is not None:
                    add_dep_helper(dc.ins, denom_last.ins, sync=False, reason="act order")
                denom_last = dc
            recip = rpool.tile([P, nb], f32, tag=f"recip{tag}", name=f"recip{tag}")
            nc.vector.reciprocal(recip[:], denom[:])
            # sacrificial same-proc read: soaks up the redundant DVE wait Tile
            # pins on the first consumer of recip
            rscr = rpool.tile([P, nb], f32, tag=f"rscr{tag}", name=f"rscr{tag}")
            nc.vector.tensor_copy(rscr[:], recip[:])
            for k, i in enumerate(banks):
                o = otile[:, i * F : (i + 1) * F]
                nc.vector.tensor_scalar_mul(o, acc[i][:, 0:F], recip[:, k : k + 1])
                last_dve = nc.vector.tensor_scalar_max(o, o, 0.0)
                stores.append(nc.gpsimd.dma_start(out[i * P : (i + 1) * P, :], o))

        epi(list(range(2, NI)), "A")
        epi([0, 1], "B")

        # Funnel every proc's final tick into SP via single-wait nops so the
        # kernel-tail drain (which otherwise aggregates ~19 sem waits, far
        # over walrus's cap) has nothing left to wait on.
        for dep in [*refills[-NBUF:], *stores, last_mm, denom_last, last_dve]:
            nop = nc.sync.nop(nofuse=True, hint="tail_funnel")
            add_dep_helper(nop.ins, dep.ins, reason="tail funnel")


def _prep_inputs(node_feats, Ahat, w, w_a, a):
    node_feats = np.asarray(node_feats, dtype=np.float32)
    Ahat = np.asarray(Ahat, dtype=np.float32)
    w = np.asarray(w, dtype=np.float32)
    w_a = np.asarray(w_a, dtype=np.float32)
    a = np.asarray(a, dtype=np.float32)

    u = w @ (w_a @ a[2:4])  # [256, 1]
    params = np.concatenate([w, u, node_feats.T], axis=1).astype("bfloat16")
    params = np.ascontiguousarray(params)  # [256, 257 + 8192]

    in_maps = []
    for c in range(NCORES):
        aT_c = np.ascontiguousarray(
            Ahat[c * ROWS : (c + 1) * ROWS, :].T.astype("bfloat16")
        )
        in_maps.append({"aT": aT_c, "params": params})
    return in_maps


def _run(inputs, trace=False, **kwargs):
    if "nc" not in _CACHE:
        _CACHE["nc"] = _build()
    nc = _CACHE["nc"]
    in_maps = _prep_inputs(**inputs)
    res = run_bass_kernel_spmd(
        nc, in_maps, core_ids=list(range(NCORES)), trace=trace, **kwargs
    )
    full = np.concatenate([res.results[c]["out"] for c in range(NCORES)], axis=0)
    return full, res


def kernel(**inputs) -> np.ndarray:
    out, _ = _run(inputs, trace=False)
    return out


# revision 58
# speedup vs baseline: 1.0242x; 1.0242x over previous
"""GAT/GCN message-passing layer on 8 Trainium2 NeuronCores.

Math: the reference computes, per query node i,
    e[i,j]   = f_src[i] + f_dst[j]           (GAT additive attention, masked by Ahat>0)
    attn     = softmax_j(e masked)
    out      = relu(attn @ h_prime)
The f_src[i] term is constant along the softmax axis, so it cancels:
    attn[i,j] = Ahat[i,j]*exp(f_dst[j]) / sum_k Ahat[i,k]*exp(f_dst[k])
With g = exp(f_dst) the whole layer is one GEMM over the adjacency:
    out = relu( (Ahat @ [g*h_prime | g])[:, :256] / (Ahat @ [g*h_prime | g])[:, 256:] )
where h_prime = node_feats @ w and f_dst = node_feats @ (w @ w_a @ a[2:4]).

Sharding: 1D row partition of query nodes.  Each of the 8 cores owns 1024 rows
of Ahat (fed pre-transposed, [8192, 1024], so the contraction axis j lands on
SBUF partitions) and computes its 1024x256 slice of the output.  The small
tensors (node_feats^T, [w | u]) are replicated; each core recomputes the
B = [g*h_prime | g] panel locally, which is cheaper than a collective.

The adjacency is binary so bf16 is lossless for it; everything else is bf16 in
/ fp32-PSUM accumulate (measured end-to-end rel-err ~2e-3 vs fp32 reference).

A quirk this kernel works around everywhere: this walrus accepts only ONE sync
wait per instruction, so the dataflow is arranged so every instruction has at
most one cross-engine dependency (single-engine op chains, ACT-issued DMAs
that inherit the scalar sequencer's observed clock, and read/write "absorber"
ops ahead of DMAs that would otherwise need two waits).
"""

import os
import sys

import numpy as np

sys.path.insert(0, "/opt/trn_rl_repo")

import concourse.bass as bass  # noqa: E402
import concourse.tile as tile  # noqa: E402
from concourse import mybir  # noqa: E402
from concourse.bass_utils import run_bass_kernel_spmd  # noqa: E402
from concourse.tile import add_dep_helper  # noqa: E402

N = 8192
F = 256  # in_features == out_features
FE = F + 1  # h_prime columns + the g column
NCORES = 8
ROWS = N // NCORES  # 1024 output rows per core
P = 128
NJ = N // P  # 64 contraction blocks
NI = ROWS // P  # 8 output-row blocks per core

BF = mybir.dt.bfloat16

# j-block layout of the adjacency stream:
#   pinned tiles: j-blocks 0..JSTART-1 stay resident (PJ j-blocks per tile)
#   rotating ring: j-blocks JSTART..63 through NBUF slots (CJ j-blocks per DMA)
PJ = 8
NPIN = 4
JSTART = NPIN * PJ  # 32
CJ = 2
NROT = (NJ - JSTART) // CJ  # 16
NBUF = 8

_CACHE = {}


def _build():
    nc = bass.Bass(
        "TRN2",
        target_bir_lowering=False,
        debug=False,
        enable_asserts=True,
        num_devices=NCORES,
    )
    aT = nc.dram_tensor("aT", [N, ROWS], BF, kind="ExternalInput").ap()
    # params = [w | u | node_feats^T], all bf16, loaded in column chunks
    params = nc.dram_tensor("params", [F, FE + N], BF, kind="ExternalInput").ap()
    out = nc.dram_tensor("out", [ROWS, F], mybir.dt.float32, kind="ExternalOutput").ap()

    with tile.TileContext(nc) as tc:
        _body(tc, aT, params, out)
    return nc


def _body(tc, aT, params, out):
    nc = tc.nc
    f32 = mybir.dt.float32
    Exp = mybir.ActivationFunctionType.Exp

    with (
        tc.tile_pool(name="consts", bufs=1) as consts,
        tc.tile_pool(name="apool", bufs=1) as apool,
        tc.tile_pool(name="gpool", bufs=4) as gpool,
        tc.tile_pool(name="opool", bufs=1) as opool,
        tc.tile_pool(name="rpool", bufs=8) as rpool,
        tc.tile_pool(name="psum", bufs=1, space="PSUM") as psum,
    ):
        # ---- loads ---------------------------------------------------------
        # params in NCHP column chunks, ACT-issued so the prefix matmuls'
        # dependencies (chunk DMA + PSUM-bank WAR + B reads) all collapse onto
        # the single Activation semaphore.
        params_sb = consts.tile([P, 2, FE + N], BF, tag="params")
        # 3 chunks: each ACT DMA issue costs ~1.9us of scalar-sequencer time
        # (256 descriptors), so few chunks -- but a small first chunk (wext +
        # 8 j-blocks) lets the prefix matmuls start at ~4us.
        NCHP = 3
        bounds = [0, FE + 512, FE + 4096, FE + N]
        pchunks = []
        for c in range(NCHP):
            lo, hi = bounds[c], bounds[c + 1]
            pchunks.append(
                nc.scalar.dma_start(
                    params_sb[:, :, lo:hi],
                    params[:, lo:hi].rearrange("(o p) f -> p o f", p=P),
                )
            )
        wext_sb = params_sb[:, :, 0:FE]
        nfT_sb = [params_sb[:, kb, FE : FE + N] for kb in range(2)]

        # Adjacency: 4 pinned 4MB loads (j-blocks 0..31) + rotating ring.
        # The adjacency loads (20MB) would otherwise starve the params
        # chunks on the shared SDMA fabric and delay the first prefix matmul
        # to ~16us; gate them behind the params data (not needed until ~35us).
        pinned = []
        for t in range(NPIN):
            pt = consts.tile([P, PJ * ROWS], BF, tag=f"aTp{t}", name=f"aTp{t}")
            pinned.append(pt)
            pdma = nc.sync.dma_start(
                pt.rearrange("p (o f) -> p o f", f=ROWS),
                aT[t * PJ * P : (t + 1) * PJ * P, :].rearrange(
                    "(o p) f -> p o f", p=P
                ),
            )
            add_dep_helper(pdma.ins, pchunks[0 if t == 0 else 2].ins, reason="params first")
        rot = []
        for rc in range(NBUF):
            rt = apool.tile([P, CJ * ROWS], BF, tag=f"aTr{rc % NBUF}", name=f"aTr{rc}")
            rot.append(rt)
            rdma = nc.sync.dma_start(
                rt.rearrange("p (o f) -> p o f", f=ROWS),
                aT[(JSTART + rc * CJ) * P : (JSTART + (rc + 1) * CJ) * P, :].rearrange(
                    "(o p) f -> p o f", p=P
                ),
            )
            add_dep_helper(rdma.ins, pchunks[2].ins, reason="params first")

        def a_block(j):
            """SBUF [128, 128] lhsT view of adjacency j-block, i-block i."""
            if j < JSTART:
                t = pinned[j // PJ]
                o = j % PJ
            else:
                t = rot[(j - JSTART) // CJ]
                o = (j - JSTART) % CJ
            return t, o

        # ---- PSUM accumulators --------------------------------------------
        acc = [
            psum.tile([P, FE], f32, tag=f"acc{i}", name=f"acc{i}") for i in range(NI)
        ]

        # ---- prefix: B[j] = [g*h_prime | g], all 64 j-blocks ---------------
        # h' matmuls borrow PSUM banks 0/1; those banks' accumulation groups
        # start at j=JSTART in the main stream (their earlier j-terms are
        # backfilled at the end from the pinned tiles) so the bank WAR
        # against the last B-prep doesn't stall the in-order PE queue.
        B_all = consts.tile([P, NJ * FE], BF, tag="B")
        btile = [B_all[:, j * FE : (j + 1) * FE] for j in range(NJ)]
        G = consts.tile([P, NJ], f32, tag="G")
        prev_act = None
        for j in range(NJ):
            hp = acc[j % 2]
            for kb in range(2):
                nc.tensor.matmul(
                    hp[:],
                    lhsT=nfT_sb[kb][:, j * P : (j + 1) * P],
                    rhs=wext_sb[:, kb, :],
                    start=(kb == 0),
                    stop=(kb == 1),
                )
            b = btile[j]
            gj = G[:, j : j + 1]
            ex = nc.scalar.activation(gj, hp[:, F : F + 1], Exp)
            if prev_act is not None:
                # keep B-prep in emission order on ACT; a scheduler shuffle
                # makes some of these ops pick up a second (same-sem) wait
                add_dep_helper(ex.ins, prev_act.ins, sync=False, reason="act order")
            prev_act = nc.scalar.mul(b[:, 0:F], hp[:, 0:F], gj)
            if j % 8 == 7:
                # one strided cast-copy drops this 8-group's g column into B
                c0 = j - 7
                prev_act = nc.scalar.copy(
                    B_all[:, c0 * FE + F : (j + 1) * FE : FE], G[:, c0 : j + 1]
                )

        # ---- main stream ---------------------------------------------------
        last_mm = None
        refills = []
        scr_last = None
        for j in range(NJ):
            t, o = a_block(j)
            ilist = range(NI) if j >= JSTART else range(2, NI)
            for i in ilist:
                first = j == 0 or (j == JSTART and i < 2)
                last_mm = nc.tensor.matmul(
                    acc[i][:],
                    lhsT=t[:, o * ROWS + i * P : o * ROWS + (i + 1) * P],
                    rhs=btile[j][:],
                    start=first,
                    stop=(j == NJ - 1 and i >= 2),
                )
            # ring refill bookkeeping (rotating region only)
            if j >= JSTART and (j - JSTART) % CJ == CJ - 1:
                rc = (j - JSTART) // CJ
                nxt = rc + NBUF
                if nxt < NROT:
                    at = rot[rc]
                    # single-wait absorbers: ACT read (old DMA dep) + ACT
                    # zero-write (PE readers dep); ACT-issued refill then
                    # needs only one wait.
                    head = at[:, 0:1]
                    tail = at[:, CJ * ROWS - 1 : CJ * ROWS]
                    scr = gpool.tile([P, 1], BF, tag="scr")
                    scr_last = scr
                    rd = nc.scalar.copy(scr[:], head)
                    wz = nc.scalar.mul(tail, tail, 0.0)
                    add_dep_helper(wz.ins, rd.ins, sync=False, reason="rd<wz")
                    at_new = apool.tile(
                        [P, CJ * ROWS], BF, tag=f"aTr{nxt % NBUF}", name=f"aTr{nxt}"
                    )
                    rot.append(at_new)
                    refill = nc.scalar.dma_start(
                        at_new.rearrange("p (o f) -> p o f", f=ROWS),
                        aT[
                            (JSTART + nxt * CJ) * P : (JSTART + (nxt + 1) * CJ) * P, :
                        ].rearrange("(o p) f -> p o f", p=P),
                    )
                    add_dep_helper(refill.ins, wz.ins, sync=False, reason="rf<wz")
                    refills.append(refill)

        # backfill: banks 0/1 take their j<JSTART terms from the pinned tiles
        for j in range(JSTART):
            t, o = a_block(j)
            for i in range(2):
                last_mm = nc.tensor.matmul(
                    acc[i][:],
                    lhsT=t[:, o * ROWS + i * P : o * ROWS + (i + 1) * P],
                    rhs=btile[j][:],
                    start=False,
                    stop=(j == JSTART - 1),
                )

        # ---- epilogue: out[i] = relu(acc[i][:, :F] / acc[i][:, F]) ---------
        # split into banks 2..7 (stop at main j=63, drain while the backfill
        # matmuls still run) and banks 0/1 (stop at backfill end).  ACT copies
        # the denominators into SBUF (one PE wait each, avoids DVE-reads-PSUM
        # bank deps), DVE does reciprocal / scale / relu, SWDGE stores.
        otile = opool.tile([P, NI * F], f32, tag="o")
        stores = []
        denom_last = None
        last_dve = None

        def epi(banks, tag):
            nonlocal denom_last, last_dve
            nb = len(banks)
            denom = rpool.tile([P, nb], f32, tag=f"denom{tag}", name=f"denom{tag}")
            for k, i in enumerate(banks):
                dc = nc.scalar.copy(denom[:, k : k + 1], acc[i][:, F : F + 1])
                if denom_last is not None:
                    add_dep_helper(dc.ins, denom_last.ins, sync=False, reason="act order")
                denom_last = dc
            recip = rpool.tile([P, nb], f32, tag=f"recip{tag}", name=f"recip{tag}")
            nc.vector.reciprocal(recip[:], denom[:])
            # sacrificial same-proc read: soaks up the redundant DVE wait Tile
            # pins on the first consumer of recip
            rscr = rpool.tile([P, nb], f32, tag=f"rscr{tag}", name=f"rscr{tag}")
            nc.vector.tensor_copy(rscr[:], recip[:])
            for k, i in enumerate(banks):
                o = otile[:, i * F : (i + 1) * F]
                nc.vector.tensor_scalar_mul(o, acc[i][:, 0:F], recip[:, k : k + 1])
                last_dve = nc.vector.tensor_scalar_max(o, o, 0.0)
                stores.append(nc.gpsimd.dma_start(out[i * P : (i + 1) * P, :], o))

        epi(list(range(2, NI)), "A")
        epi([0, 1], "B")

        # Funnel every proc's final tick into SP via single-wait nops so the
        # kernel-tail drain (which otherwise aggregates ~19 sem waits, far
        # over walrus's cap) has nothing left to wait on.
        for dep in [*refills[-NBUF:], *stores, last_mm, denom_last, last_dve]:
            nop = nc.sync.nop(nofuse=True, hint="tail_funnel")
            add_dep_helper(nop.ins, dep.ins, reason="tail funnel")


def _prep_inputs(node_feats, Ahat, w, w_a, a):
    node_feats = np.asarray(node_feats, dtype=np.float32)
    Ahat = np.asarray(Ahat, dtype=np.float32)
    w = np.asarray(w, dtype=np.float32)
    w_a = np.asarray(w_a, dtype=np.float32)
    a = np.asarray(a, dtype=np.float32)

    u = w @ (w_a @ a[2:4])  # [256, 1]
    params = np.concatenate([w, u, node_feats.T], axis=1).astype("bfloat16")
    params = np.ascontiguousarray(params)  # [256, 257 + 8192]

    in_maps = []
    for c in range(NCORES):
        aT_c = np.ascontiguousarray(
            Ahat[c * ROWS : (c + 1) * ROWS, :].T.astype("bfloat16")
        )
        in_maps.append({"aT": aT_c, "params": params})
    return in_maps


def _run(inputs, trace=False, **kwargs):
    if "nc" not in _CACHE:
        _CACHE["nc"] = _build()
    nc = _CACHE["nc"]
    in_maps = _prep_inputs(**inputs)
    res = run_bass_kernel_spmd(
        nc, in_maps, core_ids=list(range(NCORES)), trace=trace, **kwargs
    )
    full = np.concatenate([res.results[c]["out"] for c in range(NCORES)], axis=0)
    return full, res


def kernel(**inputs) -> np.ndarray:
    out, _ = _run(inputs, trace=False)
    return out


# revision 59
# speedup vs baseline: 1.0287x; 1.0044x over previous
"""GAT/GCN message-passing layer on 8 Trainium2 NeuronCores.

Math: the reference computes, per query node i,
    e[i,j]   = f_src[i] + f_dst[j]           (GAT additive attention, masked by Ahat>0)
    attn     = softmax_j(e masked)
    out      = relu(attn @ h_prime)
The f_src[i] term is constant along the softmax axis, so it cancels:
    attn[i,j] = Ahat[i,j]*exp(f_dst[j]) / sum_k Ahat[i,k]*exp(f_dst[k])
With g = exp(f_dst) the whole layer is one GEMM over the adjacency:
    out = relu( (Ahat @ [g*h_prime | g])[:, :256] / (Ahat @ [g*h_prime | g])[:, 256:] )
where h_prime = node_feats @ w and f_dst = node_feats @ (w @ w_a @ a[2:4]).

Sharding: 1D row partition of query nodes.  Each of the 8 cores owns 1024 rows
of Ahat (fed pre-transposed, [8192, 1024], so the contraction axis j lands on
SBUF partitions) and computes its 1024x256 slice of the output.  The small
tensors (node_feats^T, [w | u]) are replicated; each core recomputes the
B = [g*h_prime | g] panel locally, which is cheaper than a collective.

The adjacency is binary so bf16 is lossless for it; everything else is bf16 in
/ fp32-PSUM accumulate (measured end-to-end rel-err ~2e-3 vs fp32 reference).

A quirk this kernel works around everywhere: this walrus accepts only ONE sync
wait per instruction, so the dataflow is arranged so every instruction has at
most one cross-engine dependency (single-engine op chains, ACT-issued DMAs
that inherit the scalar sequencer's observed clock, and read/write "absorber"
ops ahead of DMAs that would otherwise need two waits).
"""

import os
import sys

import numpy as np

sys.path.insert(0, "/opt/trn_rl_repo")

import concourse.bass as bass  # noqa: E402
import concourse.tile as tile  # noqa: E402
from concourse import mybir  # noqa: E402
from concourse.bass_utils import run_bass_kernel_spmd  # noqa: E402
from concourse.tile import add_dep_helper  # noqa: E402

N = 8192
F = 256  # in_features == out_features
FE = F + 1  # h_prime columns + the g column
NCORES = 8
ROWS = N // NCORES  # 1024 output rows per core
P = 128
NJ = N // P  # 64 contraction blocks
NI = ROWS // P  # 8 output-row blocks per core

BF = mybir.dt.bfloat16

# j-block layout of the adjacency stream:
#   pinned tiles: j-blocks 0..JSTART-1 stay resident (PJ j-blocks per tile)
#   rotating ring: j-blocks JSTART..63 through NBUF slots (CJ j-blocks per DMA)
PJ = 8
NPIN = 4
JSTART = NPIN * PJ  # 32
CJ = 2
NROT = (NJ - JSTART) // CJ  # 16
NBUF = 8

_CACHE = {}


def _build():
    nc = bass.Bass(
        "TRN2",
        target_bir_lowering=False,
        debug=False,
        enable_asserts=True,
        num_devices=NCORES,
    )
    aT = nc.dram_tensor("aT", [N, ROWS], BF, kind="ExternalInput").ap()
    # params = [w | u | node_feats^T], all bf16, loaded in column chunks
    params = nc.dram_tensor("params", [F, FE + N], BF, kind="ExternalInput").ap()
    out = nc.dram_tensor("out", [ROWS, F], mybir.dt.float32, kind="ExternalOutput").ap()

    with tile.TileContext(nc) as tc:
        _body(tc, aT, params, out)
    return nc


def _body(tc, aT, params, out):
    nc = tc.nc
    f32 = mybir.dt.float32
    Exp = mybir.ActivationFunctionType.Exp

    with (
        tc.tile_pool(name="consts", bufs=1) as consts,
        tc.tile_pool(name="apool", bufs=1) as apool,
        tc.tile_pool(name="gpool", bufs=4) as gpool,
        tc.tile_pool(name="opool", bufs=1) as opool,
        tc.tile_pool(name="rpool", bufs=8) as rpool,
        tc.tile_pool(name="psum", bufs=1, space="PSUM") as psum,
    ):
        # ---- loads ---------------------------------------------------------
        # params in NCHP column chunks, ACT-issued so the prefix matmuls'
        # dependencies (chunk DMA + PSUM-bank WAR + B reads) all collapse onto
        # the single Activation semaphore.
        params_sb = consts.tile([P, 2, FE + N], BF, tag="params")
        # 3 chunks: each ACT DMA issue costs ~1.9us of scalar-sequencer time
        # (256 descriptors), so few chunks -- but a small first chunk (wext +
        # 8 j-blocks) lets the prefix matmuls start at ~4us.
        NCHP = 3
        bounds = [0, FE + 512, FE + 4096, FE + N]
        pchunks = []
        for c in range(NCHP):
            lo, hi = bounds[c], bounds[c + 1]
            pchunks.append(
                nc.scalar.dma_start(
                    params_sb[:, :, lo:hi],
                    params[:, lo:hi].rearrange("(o p) f -> p o f", p=P),
                )
            )
        wext_sb = params_sb[:, :, 0:FE]
        nfT_sb = [params_sb[:, kb, FE : FE + N] for kb in range(2)]

        # Adjacency: 4 pinned 4MB loads (j-blocks 0..31) + rotating ring.
        # The adjacency loads (20MB) would otherwise starve the params
        # chunks on the shared SDMA fabric and delay the first prefix matmul
        # to ~16us; gate them behind the params data (not needed until ~35us).
        pinned = []
        for t in range(NPIN):
            pt = consts.tile([P, PJ * ROWS], BF, tag=f"aTp{t}", name=f"aTp{t}")
            pinned.append(pt)
            pdma = nc.sync.dma_start(
                pt.rearrange("p (o f) -> p o f", f=ROWS),
                aT[t * PJ * P : (t + 1) * PJ * P, :].rearrange(
                    "(o p) f -> p o f", p=P
                ),
            )
            add_dep_helper(pdma.ins, pchunks[0 if t == 0 else 2].ins, reason="params first")
        rot = []
        for rc in range(NBUF):
            rt = apool.tile([P, CJ * ROWS], BF, tag=f"aTr{rc % NBUF}", name=f"aTr{rc}")
            rot.append(rt)
            rdma = nc.sync.dma_start(
                rt.rearrange("p (o f) -> p o f", f=ROWS),
                aT[(JSTART + rc * CJ) * P : (JSTART + (rc + 1) * CJ) * P, :].rearrange(
                    "(o p) f -> p o f", p=P
                ),
            )
            add_dep_helper(rdma.ins, pchunks[2].ins, reason="params first")

        def a_block(j):
            """SBUF [128, 128] lhsT view of adjacency j-block, i-block i."""
            if j < JSTART:
                t = pinned[j // PJ]
                o = j % PJ
            else:
                t = rot[(j - JSTART) // CJ]
                o = (j - JSTART) % CJ
            return t, o

        # ---- PSUM accumulators --------------------------------------------
        acc = [
            psum.tile([P, FE], f32, tag=f"acc{i}", name=f"acc{i}") for i in range(NI)
        ]

        # ---- prefix: B[j] = [g*h_prime | g], all 64 j-blocks ---------------
        # h' matmuls borrow PSUM banks 0/1; those banks' accumulation groups
        # start at j=JSTART in the main stream (their earlier j-terms are
        # backfilled at the end from the pinned tiles) so the bank WAR
        # against the last B-prep doesn't stall the in-order PE queue.
        B_all = consts.tile([P, NJ * FE], BF, tag="B")
        btile = [B_all[:, j * FE : (j + 1) * FE] for j in range(NJ)]
        G = consts.tile([P, NJ], f32, tag="G")
        prev_act = None
        for j in range(NJ):
            hp = acc[j % 2]
            for kb in range(2):
                nc.tensor.matmul(
                    hp[:],
                    lhsT=nfT_sb[kb][:, j * P : (j + 1) * P],
                    rhs=wext_sb[:, kb, :],
                    start=(kb == 0),
                    stop=(kb == 1),
                )
            b = btile[j]
            gj = G[:, j : j + 1]
            ex = nc.scalar.activation(gj, hp[:, F : F + 1], Exp)
            if prev_act is not None:
                # keep B-prep in emission order on ACT; a scheduler shuffle
                # makes some of these ops pick up a second (same-sem) wait
                add_dep_helper(ex.ins, prev_act.ins, sync=False, reason="act order")
            prev_act = nc.scalar.mul(b[:, 0:F], hp[:, 0:F], gj)
            if j % 8 == 7:
                # one strided cast-copy drops this 8-group's g column into B
                c0 = j - 7
                prev_act = nc.scalar.copy(
                    B_all[:, c0 * FE + F : (j + 1) * FE : FE], G[:, c0 : j + 1]
                )

        # ---- main stream ---------------------------------------------------
        last_mm = None
        refills = []
        scr_last = None
        for j in range(NJ):
            t, o = a_block(j)
            ilist = range(NI) if j >= JSTART else range(2, NI)
            for i in ilist:
                first = j == 0 or (j == JSTART and i < 2)
                last_mm = nc.tensor.matmul(
                    acc[i][:],
                    lhsT=t[:, o * ROWS + i * P : o * ROWS + (i + 1) * P],
                    rhs=btile[j][:],
                    start=first,
                    stop=(j == NJ - 1 and i >= 2),
                )
            # ring refill bookkeeping (rotating region only)
            if j >= JSTART and (j - JSTART) % CJ == CJ - 1:
                rc = (j - JSTART) // CJ
                nxt = rc + NBUF
                if nxt < NROT:
                    at = rot[rc]
                    # single-wait absorbers: ACT read (old DMA dep) + ACT
                    # zero-write (PE readers dep); ACT-issued refill then
                    # needs only one wait.
                    head = at[:, 0:1]
                    tail = at[:, CJ * ROWS - 1 : CJ * ROWS]
                    scr = gpool.tile([P, 1], BF, tag="scr")
                    scr_last = scr
                    rd = nc.scalar.copy(scr[:], head)
                    wz = nc.scalar.mul(tail, tail, 0.0)
                    add_dep_helper(wz.ins, rd.ins, sync=False, reason="rd<wz")
                    at_new = apool.tile(
                        [P, CJ * ROWS], BF, tag=f"aTr{nxt % NBUF}", name=f"aTr{nxt}"
                    )
                    rot.append(at_new)
                    refill = nc.scalar.dma_start(
                        at_new.rearrange("p (o f) -> p o f", f=ROWS),
                        aT[
                            (JSTART + nxt * CJ) * P : (JSTART + (nxt + 1) * CJ) * P, :
                        ].rearrange("(o p) f -> p o f", p=P),
                    )
                    add_dep_helper(refill.ins, wz.ins, sync=False, reason="rf<wz")
                    refills.append(refill)

        # backfill: banks 0/1 take their j<JSTART terms from the pinned tiles
        for j in range(JSTART):
            t, o = a_block(j)
            for i in range(2):
                last_mm = nc.tensor.matmul(
                    acc[i][:],
                    lhsT=t[:, o * ROWS + i * P : o * ROWS + (i + 1) * P],
                    rhs=btile[j][:],
                    start=False,
                    stop=(j == JSTART - 1),
                )

        # ---- epilogue: out[i] = relu(acc[i][:, :F] / acc[i][:, F]) ---------
        # split into banks 2..7 (stop at main j=63, drain while the backfill
        # matmuls still run) and banks 0/1 (stop at backfill end).  ACT copies
        # the denominators into SBUF (one PE wait each, avoids DVE-reads-PSUM
        # bank deps), DVE does reciprocal / scale / relu, SWDGE stores.
        otile = opool.tile([P, NI * F], f32, tag="o")
        stores = []
        denom_last = None
        last_dve = None
        last_relu = None

        # banks 2..7: no prefix ACT history on their PSUM banks, so ACT can
        # read them directly -- one fused relu(acc*recip) activation per bank,
        # running in parallel with the DVE path for banks 0/1 below.
        banksA = list(range(2, NI))
        denomA = rpool.tile([P, len(banksA)], f32, tag="denomA")
        for k, i in enumerate(banksA):
            dc = nc.scalar.copy(denomA[:, k : k + 1], acc[i][:, F : F + 1])
            if denom_last is not None:
                add_dep_helper(dc.ins, denom_last.ins, sync=False, reason="act order")
            denom_last = dc
        recipA = rpool.tile([P, len(banksA)], f32, tag="recipA")
        nc.vector.reciprocal(recipA[:], denomA[:])
        # sacrificial ACT read: absorbs recipA's DVE tick for all six relus
        sacA = rpool.tile([P, len(banksA)], f32, tag="sacA")
        sa = nc.scalar.copy(sacA[:], recipA[:])
        add_dep_helper(sa.ins, denom_last.ins, sync=False, reason="act order")
        last_relu = sa
        for k, i in enumerate(banksA):
            o = otile[:, i * F : (i + 1) * F]
            rl = nc.scalar.activation(
                o,
                acc[i][:, 0:F],
                mybir.ActivationFunctionType.Relu,
                scale=recipA[:, k : k + 1],
            )
            add_dep_helper(rl.ins, last_relu.ins, sync=False, reason="act order")
            last_relu = rl
            stores.append(nc.gpsimd.dma_start(out[i * P : (i + 1) * P, :], o))

        # banks 0/1 (prefix-touched): DVE path as before
        denomB = rpool.tile([P, 2], f32, tag="denomB")
        for k, i in enumerate([0, 1]):
            dc = nc.scalar.copy(denomB[:, k : k + 1], acc[i][:, F : F + 1])
            add_dep_helper(dc.ins, (denom_last or dc).ins, sync=False, reason="act order")
            denom_last = dc
        recipB = rpool.tile([P, 2], f32, tag="recipB")
        nc.vector.reciprocal(recipB[:], denomB[:])
        rscrB = rpool.tile([P, 2], f32, tag="rscrB")
        nc.vector.tensor_copy(rscrB[:], recipB[:])
        for k, i in enumerate([0, 1]):
            o = otile[:, i * F : (i + 1) * F]
            nc.vector.tensor_scalar_mul(o, acc[i][:, 0:F], recipB[:, k : k + 1])
            last_dve = nc.vector.tensor_scalar_max(o, o, 0.0)
            stores.append(nc.gpsimd.dma_start(out[i * P : (i + 1) * P, :], o))

        # Funnel every proc's final tick into SP via single-wait nops so the
        # kernel-tail drain (which otherwise aggregates ~19 sem waits, far
        # over walrus's cap) has nothing left to wait on.
        for dep in [*refills[-NBUF:], *stores, last_mm, last_relu, last_dve]:
            nop = nc.sync.nop(nofuse=True, hint="tail_funnel")
            add_dep_helper(nop.ins, dep.ins, reason="tail funnel")


def _prep_inputs(node_feats, Ahat, w, w_a, a):
    node_feats = np.asarray(node_feats, dtype=np.float32)
    Ahat = np.asarray(Ahat, dtype=np.float32)
    w = np.asarray(w, dtype=np.float32)
    w_a = np.asarray(w_a, dtype=np.float32)
    a = np.asarray(a, dtype=np.float32)

    u = w @ (w_a @ a[2:4])  # [256, 1]
    params = np.concatenate([w, u, node_feats.T], axis=1).astype("bfloat16")
    params = np.ascontiguousarray(params)  # [256, 257 + 8192]

    in_maps = []
    for c in range(NCORES):
        aT_c = np.ascontiguousarray(
            Ahat[c * ROWS : (c + 1) * ROWS, :].T.astype("bfloat16")
        )
        in_maps.append({"aT": aT_c, "params": params})
    return in_maps


def _run(inputs, trace=False, **kwargs):
    if "nc" not in _CACHE:
        _CACHE["nc"] = _build()
    nc = _CACHE["nc"]
    in_maps = _prep_inputs(**inputs)
    res = run_bass_kernel_spmd(
        nc, in_maps, core_ids=list(range(NCORES)), trace=trace, **kwargs
    )
    full = np.concatenate([res.results[c]["out"] for c in range(NCORES)], axis=0)
    return full, res


def kernel(**inputs) -> np.ndarray:
    out, _ = _run(inputs, trace=False)
    return out
